# revision 38
# baseline (speedup 1.0000x reference)
"""Multi-head attention (S=2048, B=2, E=1024, H=16, D=64) on 8 Trainium2 cores.

Sharding: batch*heads head-parallel. Core c owns heads {2c, 2c+1} for both
batch elements (4 of the 32 (b,h) attention pairs). Host-side sharding prep:
slice/scale/cast the in_proj weights per core, cast x to bf16 and lay it out
as xT tiles [B, KT, 128, S] (contraction dim on partitions), and concatenate
the per-core outputs along E. All FLOPs run on device. Per core:
  1. Load xT / weight tiles over all three DMA queues (sync/scalar/gpsimd).
  2. Project q,k,v, all transposed ([col, tok]; col = 2 heads x 64) as
     out = W_chunk.T @ xT with fp32 PSUM accumulation over 8 E-tiles; bias
     added during DVE evacuation (q and its bias pre-scaled by D^-0.5).
     v is then PE-transposed per 128-kpos tile into va [128, kt, 130] =
     [v_h0 | 1 | v_h1 | 1] (ones from a memset, giving softmax denominators).
  3. Attention per (b, q-chunk of 512): for each kpos tile, both heads'
     score tiles go into one [128, 1024] PSUM tile (lhsT = kT slice, K=64;
     alternating PE row groups 0/64 so each LDWEIGHTS hides under the other
     head's matmul); one ScalarE Exp evacuates PSUM->SBUF bf16 [128, 1024];
     then attnT[h] [65, 512] accumulates va_slice.T @ exp over all 16 kpos
     tiles (row 64 = sum of exp, the softmax denominator).
  4. Normalize: DVE-copy attnT to SBUF, PE-transpose [65,128] blocks back to
     [128 q, 65], DVE reciprocal of col 64 and per-partition scalar multiply
     into the output gather tile; one DMA per (b, q-chunk) writes DRAM.
Emission-order matters for Tile deps: consumers must be emitted after
their producers (attend(0,0) is split into scores/exp then attT phases so
the pumped v-projection lands in between). A PE warm-up burst beats the
HAM cold clock at kernel start.
Each attend defers its normalize tail (PE transposes + reciprocal/mul)
into the next attend's kt loop as pumped thunks so the tail never stalls
ScalarE between q-chunks.
Measured on trn2: ~224-229 us HW exec (288 us first working version),
rel err ~3.8e-3 vs the fp32 reference.
"""

import numpy as np
import ml_dtypes

S, B, E = 2048, 2, 1024
H, D = 16, 64
SCALING = D ** -0.5
NCORES = 8
SB = S * B            # 4096 tokens, row = s*B + b
HPC = H // NCORES     # 2 heads per core
KT = E // 128         # 8 contraction tiles over E
QCHUNK = 512
NQC = S // QCHUNK     # 4 q-chunks
NKT = S // 128        # 16 kpos tiles
VN = 2 * (D + 1)      # 130 va cols: [v_h0(64) | 1 | v_h1(64) | 1]

_BF16 = ml_dtypes.bfloat16
_BUILT = {}


def _build_bass():
    import concourse.bacc as bacc
    import concourse.mybir as mybir
    import concourse.tile as tile
    from contextlib import ExitStack

    f32 = mybir.dt.float32
    bf = mybir.dt.bfloat16

    nc = bacc.Bacc(None, target_bir_lowering=False, debug=False)

    xt_in = nc.dram_tensor("xt", [B, KT, 128, S], bf, kind="ExternalInput")
    wqkv_in = nc.dram_tensor("wqkv", [E, 384], bf, kind="ExternalInput")
    bqkv_in = nc.dram_tensor("bqkv", [384, 1], f32, kind="ExternalInput")
    id64_in = nc.dram_tensor("id64", [128, 64], bf, kind="ExternalInput")
    id65_in = nc.dram_tensor("id65", [65, 65], f32, kind="ExternalInput")
    out_d = nc.dram_tensor("out", [S, B, 2 * D], f32, kind="ExternalOutput")

    with tile.TileContext(nc) as tc, ExitStack() as ctx:
        const = ctx.enter_context(tc.tile_pool(name="const", bufs=1))
        res = ctx.enter_context(tc.tile_pool(name="res", bufs=1))
        expp = ctx.enter_context(tc.tile_pool(name="expp", bufs=16))
        atn = ctx.enter_context(tc.tile_pool(name="atn", bufs=3))
        ogp = ctx.enter_context(tc.tile_pool(name="ogp", bufs=8))
        rp = ctx.enter_context(tc.tile_pool(name="rp", bufs=8))
        ps_sc = ctx.enter_context(tc.tile_pool(name="ps_sc", bufs=2, space="PSUM"))
        ps_sm = ctx.enter_context(tc.tile_pool(name="ps_sm", bufs=4, space="PSUM"))

        # ---- constants ----
        wqkv_sb = [const.tile([128, 384], bf, tag=f"wqkv{k}", name=f"wqkv{k}") for k in range(KT)]
        for k in range(KT):
            eng = (nc.sync, nc.scalar, nc.gpsimd)[(k + 1) % 3]
            eng.dma_start(out=wqkv_sb[k][:], in_=wqkv_in[k * 128:(k + 1) * 128, :])
        bqkv_sb = const.tile([128, 3], f32, tag="bqkv")
        nc.gpsimd.dma_start(
            out=bqkv_sb[:], in_=bqkv_in.rearrange("(c p) o -> p (c o)", p=128)
        )
        id64 = const.tile([128, 64], bf, tag="id64")
        nc.sync.dma_start(out=id64[:], in_=id64_in[:])
        id65 = const.tile([65, 65], f32, tag="id65")
        nc.scalar.dma_start(out=id65[:], in_=id65_in[:])

        # ---- x^T tiles (host pre-transposed as sharding prep) ----
        # two 2MB DMAs per batch on parallel queues: near line-rate head
        xTg = [
            [res.tile([128, 4, S], bf, tag=f"xTg{b}_{g}", name=f"xTg{b}_{g}") for g in range(2)]
            for b in range(B)
        ]
        x4 = xt_in.rearrange("b (g k) p s -> b g k p s", g=2)
        for b in range(B):
            for g in range(2):
                for kk in range(4):
                    eng = (nc.sync, nc.scalar, nc.gpsimd)[(g * 4 + kk) % 3]
                    eng.dma_start(
                        out=xTg[b][g][:, kk, :], in_=x4[b, g, kk]
                    )

        def xs(b, k):
            return xTg[b][k // 4][:, k % 4, :]

        warm = ps_sm.tile([128, 384], f32, tag="ps1", name="warm")
        for _ in range(18):
            nc.tensor.matmul(
                warm[:], lhsT=wqkv_sb[0][:, 0:128], rhs=wqkv_sb[0][:],
                start=True, stop=True,
            )

        qT = [res.tile([128, S], bf, tag=f"qT{b}", name=f"qTt{b}") for b in range(B)]
        kT = [res.tile([128, S], bf, tag=f"kT{b}", name=f"kTt{b}") for b in range(B)]
        vT = [res.tile([128, S], bf, tag=f"vT{b}", name=f"vTt{b}") for b in range(B)]
        va = [res.tile([128, NKT, VN], bf, tag=f"va{b}", name=f"vat{b}") for b in range(B)]
        for b in range(B):
            nc.vector.memset(va[b][:], 1.0)  # ones cols survive at 64, 129

        def proj_block(b, which):
            # out[col, tok] for col-chunk `which` (0=q, 1=k, 2=v)
            dst = (qT[b], kT[b], vT[b])[which]
            for t in range(NQC):
                ps = ps_sm.tile([128, QCHUNK], f32, tag="ps1", name="projps")
                for k in range(KT):
                    nc.tensor.matmul(
                        ps[:],
                        lhsT=wqkv_sb[k][:, which * 128:(which + 1) * 128],
                        rhs=xs(b, k)[:, t * QCHUNK:(t + 1) * QCHUNK],
                        start=(k == 0),
                        stop=(k == KT - 1),
                    )
                nc.vector.tensor_scalar_add(
                    out=dst[:, t * QCHUNK:(t + 1) * QCHUNK],
                    in0=ps[:],
                    scalar1=bqkv_sb[:, which:which + 1],
                )

        def v_transposes(b, t):
            # vT chunk t covers kpos tiles 4t..4t+3; transpose to va natural
            for kt in range(4 * t, 4 * t + 4):
                for h in range(HPC):
                    pst = ps_sm.tile([128, 64], bf, tag="ps1", name="vtps")
                    nc.tensor.transpose(
                        pst[:],
                        in_=vT[b][h * 64:(h + 1) * 64, kt * 128:(kt + 1) * 128],
                        identity=id64[h * 64:(h + 1) * 64, :],
                    )
                    nc.vector.tensor_copy(
                        out=va[b][:, kt, h * (D + 1):h * (D + 1) + D], in_=pst[:]
                    )

        def proj_pump(b, items):
            # yields after each emitted op so attend() can interleave
            # projection work into PE idle slots of the ACT-paced attention
            for which, t in items:
                dst = (qT[b], kT[b], vT[b])[which]
                if True:
                    ps = ps_sm.tile([128, QCHUNK], f32, tag="ps1", name="projps")
                    for k in range(KT):
                        nc.tensor.matmul(
                            ps[:],
                            lhsT=wqkv_sb[k][:, which * 128:(which + 1) * 128],
                            rhs=xs(b, k)[:, t * QCHUNK:(t + 1) * QCHUNK],
                            start=(k == 0),
                            stop=(k == KT - 1),
                        )
                        yield
                    nc.vector.tensor_scalar_add(
                        out=dst[:, t * QCHUNK:(t + 1) * QCHUNK],
                        in0=ps[:],
                        scalar1=bqkv_sb[:, which:which + 1],
                    )
                    yield
                    if which == 2:
                        for kt in range(4 * t, 4 * t + 4):
                            for h in range(HPC):
                                pst = ps_sm.tile([128, 64], bf, tag="ps1", name="vtps")
                                nc.tensor.transpose(
                                    pst[:],
                                    in_=vT[b][h * 64:(h + 1) * 64, kt * 128:(kt + 1) * 128],
                                    identity=id64[h * 64:(h + 1) * 64, :],
                                )
                                nc.vector.tensor_copy(
                                    out=va[b][:, kt, h * (D + 1):h * (D + 1) + D],
                                    in_=pst[:],
                                )
                            yield

        def _norm_unit(att_sb, og, h, qs):
            pst = ps_sm.tile([128, D + 1], f32, tag="ps1", name="attt")
            nc.tensor.transpose(
                pst[:], in_=att_sb[:, qs * 128:(qs + 1) * 128], identity=id65[:]
            )
            rec = rp.tile([128, 1], f32, tag="rec", name="rec")
            nc.vector.reciprocal(out=rec[:], in_=pst[:, D:D + 1])
            nc.vector.tensor_scalar_mul(
                out=og[:, qs, h * D:(h + 1) * D], in0=pst[:, 0:D], scalar1=rec[:]
            )

        def attend(b, qc, pump=None, rate=3, pending=None, last=False):
            og = ogp.tile([128, 4, 2 * D], f32, tag="og", name="og")
            # attn^T accumulators per head: rows 0-63 = dims, row 64 = sum(exp)
            att = [
                ps_sm.tile([D + 1, QCHUNK], f32, tag="ps1", name="attps")
                for _ in range(HPC)
            ]
            qsl = qT[b][:, qc * QCHUNK:(qc + 1) * QCHUNK]
            for kt in range(NKT):
                # both heads' scores for this kpos tile in one 2-bank tile;
                # alternating PE row groups (0 / 64) hide each LDWEIGHTS
                # under the other head's matmul
                sc = ps_sc.tile([128, 1024], f32, tag="sc", name="scps")
                for h in range(HPC):
                    nc.tensor.matmul(
                        sc[:, h * 512:(h + 1) * 512],
                        lhsT=kT[b][h * 64:(h + 1) * 64, kt * 128:(kt + 1) * 128],
                        rhs=qsl[h * 64:(h + 1) * 64, :],
                        start=True,
                        stop=True,
                    )
                ex = expp.tile([128, 1024], bf, tag="ex", name="ex")
                nc.scalar.activation(
                    out=ex[:], in_=sc[:], func=mybir.ActivationFunctionType.Exp
                )
                for h in range(HPC):
                    nc.tensor.matmul(
                        att[h][:],
                        lhsT=va[b][:, kt, h * (D + 1):(h + 1) * (D + 1)],
                        rhs=ex[:, h * 512:(h + 1) * 512],
                        start=(kt == 0),
                        stop=(kt == NKT - 1),
                    )
                if pending:
                    for _ in range(2):
                        if pending:
                            pending.pop(0)()
                if pump is not None:
                    for _ in range(rate):
                        next(pump, None)
            while pending:
                pending.pop(0)()
            # release the PSUM accumulators now; defer the PE/DVE normalize
            # tail into the next attend's kt loop so it doesn't stall ScalarE
            new_pending = []
            for h in range(HPC):
                att_sb = atn.tile([D + 1, QCHUNK], f32, tag="atn", name="attsb")
                nc.vector.tensor_copy(out=att_sb[:], in_=att[h][:])
                for qs in range(4):
                    new_pending.append(
                        lambda a=att_sb, hh=h, q=qs: _norm_unit(a, og, hh, q)
                    )
            new_pending.append(
                lambda: nc.gpsimd.dma_start(
                    out=out_d.rearrange("(qs p) b e -> p qs b e", p=128)[
                        :, qc * 4:(qc + 1) * 4, b, :
                    ],
                    in_=og[:],
                )
            )
            if last:
                for th in new_pending:
                    th()
                return []
            return new_pending

        # b0: only k and q projections up front (all the exp chain needs);
        # v projection + transposes dribble into attend(0,0)'s scores/exp
        # phase so ScalarE starts ~25us earlier. Tile deps are emission-
        # ordered, so attend(0,0)'s attT matmuls (which consume va) are
        # emitted only AFTER the v pump is fully drained.
        proj_block(0, 1)
        # qT chunk 0 only; chunks 1-3 ride the pump (consumed ~15us later
        # by attend(0,1..3), so they have ample runtime lead)
        ps00 = ps_sm.tile([128, QCHUNK], f32, tag="ps1", name="projps")
        for k in range(KT):
            nc.tensor.matmul(
                ps00[:], lhsT=wqkv_sb[k][:, 0:128], rhs=xs(0, k)[:, 0:QCHUNK],
                start=(k == 0), stop=(k == KT - 1),
            )
        nc.vector.tensor_scalar_add(
            out=qT[0][:, 0:QCHUNK], in0=ps00[:], scalar1=bqkv_sb[:, 0:1]
        )
        pump0 = proj_pump(
            0, [(0, 1), (0, 2), (0, 3)] + [(2, t) for t in range(NQC)]
        )
        og0 = ogp.tile([128, 4, 2 * D], f32, tag="og", name="og")
        att0 = [
            ps_sm.tile([D + 1, QCHUNK], f32, tag="ps1", name="attps")
            for _ in range(HPC)
        ]
        exs0 = []
        for kt in range(NKT):
            sc = ps_sc.tile([128, 1024], f32, tag="sc", name="scps")
            for h in range(HPC):
                nc.tensor.matmul(
                    sc[:, h * 512:(h + 1) * 512],
                    lhsT=kT[0][h * 64:(h + 1) * 64, kt * 128:(kt + 1) * 128],
                    rhs=qT[0][h * 64:(h + 1) * 64, 0:QCHUNK],
                    start=True,
                    stop=True,
                )
            ex = expp.tile([128, 1024], bf, tag="ex", name="ex")
            nc.scalar.activation(
                out=ex[:], in_=sc[:], func=mybir.ActivationFunctionType.Exp
            )
            exs0.append(ex)
            for _ in range(5):
                next(pump0, None)
        for _ in pump0:
            pass
        for kt in range(NKT):
            for h in range(HPC):
                nc.tensor.matmul(
                    att0[h][:],
                    lhsT=va[0][:, kt, h * (D + 1):(h + 1) * (D + 1)],
                    rhs=exs0[kt][:, h * 512:(h + 1) * 512],
                    start=(kt == 0),
                    stop=(kt == NKT - 1),
                )
        pend = []
        for h in range(HPC):
            att_sb = atn.tile([D + 1, QCHUNK], f32, tag="atn", name="attsb")
            nc.vector.tensor_copy(out=att_sb[:], in_=att0[h][:])
            for qs in range(4):
                pend.append(lambda a=att_sb, hh=h, q=qs: _norm_unit(a, og0, hh, q))
        pend.append(
            lambda: nc.gpsimd.dma_start(
                out=out_d.rearrange("(qs p) b e -> p qs b e", p=128)[:, 0:4, 0, :],
                in_=og0[:],
            )
        )
        # b1 projection dribbled into the remaining attend(b0) chunks;
        # each attend's normalize tail is deferred into the next attend
        pump = proj_pump(1, [(w, t) for w in (2, 1, 0) for t in range(NQC)])
        for qc in range(1, NQC):
            pend = attend(0, qc, pump, pending=pend)
        for _ in pump:
            pass
        for qc in range(NQC):
            pend = attend(1, qc, pending=pend, last=(qc == NQC - 1))

    nc.compile()
    return nc


def _get_nc():
    if "nc" not in _BUILT:
        _BUILT["nc"] = _build_bass()
    return _BUILT["nc"]


def _prep_core_inputs(x_bf, W, b):
    """Per-core input dicts. W/b slicing+scaling+casting is host-side weight prep."""
    _id64 = np.concatenate([np.eye(64), np.eye(64)], axis=0).astype(_BF16)
    _id65 = np.eye(65, dtype=np.float32)
    in_maps = []
    for c in range(NCORES):
        q0 = 2 * c * D          # first col of this core's head pair
        wq = W[:, q0:q0 + 128] * SCALING
        wk = W[:, E + q0:E + q0 + 128]
        wv = W[:, 2 * E + q0:2 * E + q0 + 128]
        wqkv = np.concatenate([wq, wk, wv], axis=1).astype(_BF16)
        bqkv = np.concatenate(
            [b[q0:q0 + 128] * SCALING, b[E + q0:E + q0 + 128],
             b[2 * E + q0:2 * E + q0 + 128]]
        ).astype(np.float32)[:, None]
        in_maps.append(
            {
                "xt": x_bf,
                "wqkv": np.ascontiguousarray(wqkv),
                "bqkv": np.ascontiguousarray(bqkv),
                "id64": _id64,
                "id65": _id65,
            }
        )
    return in_maps


def run(inputs, trace=False):
    """Returns (output [S,B,E] fp32, BassKernelResults)."""
    from concourse.bass_utils import run_bass_kernel_spmd

    x = np.asarray(inputs["x"], np.float32)
    W = np.asarray(inputs["W_in"], np.float32)
    b = np.asarray(inputs["b_in"], np.float32)
    # sharding prep: cast + de-interleave batches + transpose to [B, KT, 128, S]
    x_bf = np.ascontiguousarray(
        x.reshape(S, B, KT, 128).transpose(1, 2, 3, 0)
    ).astype(_BF16)

    nc = _get_nc()
    in_maps = _prep_core_inputs(x_bf, W, b)
    res = run_bass_kernel_spmd(
        nc, in_maps, core_ids=list(range(NCORES)), trace=trace
    )
    out = np.concatenate([r["out"] for r in res.results], axis=2)
    return out, res


def kernel(**inputs):
    out, _ = run(inputs, trace=False)
    return out


# revision 39
# speedup vs baseline: 1.0406x; 1.0406x over previous
"""Multi-head attention (S=2048, B=2, E=1024, H=16, D=64) on 8 Trainium2 cores.

Sharding: batch*heads head-parallel. Core c owns heads {2c, 2c+1} for both
batch elements (4 of the 32 (b,h) attention pairs). Host-side sharding prep:
slice/scale/cast the in_proj weights per core, cast x to bf16 and lay it out
as xT tiles [B, KT, 128, S] (contraction dim on partitions), and concatenate
the per-core outputs along E. All FLOPs run on device. Per core:
  1. Load xT / weight tiles over all three DMA queues (sync/scalar/gpsimd).
  2. Project q,k,v, all transposed ([col, tok]; col = 2 heads x 64) as
     out = W_chunk.T @ xT with fp32 PSUM accumulation over 8 E-tiles; bias
     added during DVE evacuation (q and its bias pre-scaled by D^-0.5).
     v is then PE-transposed per 128-kpos tile into va [128, kt, 130] =
     [v_h0 | 1 | v_h1 | 1] (ones from a memset, giving softmax denominators).
  3. Attention per (b, q-chunk of 512): for each kpos tile, both heads'
     score tiles go into one [128, 1024] PSUM tile (lhsT = kT slice, K=64;
     alternating PE row groups 0/64 so each LDWEIGHTS hides under the other
     head's matmul); one ScalarE Exp evacuates PSUM->SBUF bf16 [128, 1024];
     then attnT[h] [65, 512] accumulates va_slice.T @ exp over all 16 kpos
     tiles (row 64 = sum of exp, the softmax denominator).
  4. Normalize: DVE-copy attnT to SBUF, PE-transpose [65,128] blocks back to
     [128 q, 65], DVE reciprocal of col 64 and per-partition scalar multiply
     into the output gather tile; one DMA per (b, q-chunk) writes DRAM.
Emission-order matters for Tile deps: consumers must be emitted after
their producers (attend(0,0) is split into scores/exp then attT phases so
the pumped v-projection lands in between). A PE warm-up burst beats the
HAM cold clock at kernel start.
Measured on trn2: ~236 us HW exec (288 us first working version), rel err
~3.8e-3 vs the fp32 reference.
"""

import numpy as np
import ml_dtypes

S, B, E = 2048, 2, 1024
H, D = 16, 64
SCALING = D ** -0.5
NCORES = 8
SB = S * B            # 4096 tokens, row = s*B + b
HPC = H // NCORES     # 2 heads per core
KT = E // 128         # 8 contraction tiles over E
QCHUNK = 512
NQC = S // QCHUNK     # 4 q-chunks
NKT = S // 128        # 16 kpos tiles
VN = 2 * (D + 1)      # 130 va cols: [v_h0(64) | 1 | v_h1(64) | 1]

_BF16 = ml_dtypes.bfloat16
_BUILT = {}


def _build_bass():
    import concourse.bacc as bacc
    import concourse.mybir as mybir
    import concourse.tile as tile
    from contextlib import ExitStack

    f32 = mybir.dt.float32
    bf = mybir.dt.bfloat16

    nc = bacc.Bacc(None, target_bir_lowering=False, debug=False)

    xt_in = nc.dram_tensor("xt", [B, KT, 128, S], bf, kind="ExternalInput")
    wqkv_in = nc.dram_tensor("wqkv", [E, 384], bf, kind="ExternalInput")
    bqkv_in = nc.dram_tensor("bqkv", [384, 1], f32, kind="ExternalInput")
    id64_in = nc.dram_tensor("id64", [128, 64], bf, kind="ExternalInput")
    id65_in = nc.dram_tensor("id65", [65, 65], f32, kind="ExternalInput")
    out_d = nc.dram_tensor("out", [S, B, 2 * D], f32, kind="ExternalOutput")

    with tile.TileContext(nc) as tc, ExitStack() as ctx:
        const = ctx.enter_context(tc.tile_pool(name="const", bufs=1))
        res = ctx.enter_context(tc.tile_pool(name="res", bufs=1))
        expp = ctx.enter_context(tc.tile_pool(name="expp", bufs=16))
        atn = ctx.enter_context(tc.tile_pool(name="atn", bufs=3))
        ogp = ctx.enter_context(tc.tile_pool(name="ogp", bufs=8))
        rp = ctx.enter_context(tc.tile_pool(name="rp", bufs=8))
        ps_sc = ctx.enter_context(tc.tile_pool(name="ps_sc", bufs=2, space="PSUM"))
        ps_sm = ctx.enter_context(tc.tile_pool(name="ps_sm", bufs=4, space="PSUM"))

        # ---- constants ----
        wqkv_sb = [const.tile([128, 384], bf, tag=f"wqkv{k}", name=f"wqkv{k}") for k in range(KT)]
        for k in range(KT):
            eng = (nc.sync, nc.scalar, nc.gpsimd)[(k + 1) % 3]
            eng.dma_start(out=wqkv_sb[k][:], in_=wqkv_in[k * 128:(k + 1) * 128, :])
        bqkv_sb = const.tile([128, 3], f32, tag="bqkv")
        nc.gpsimd.dma_start(
            out=bqkv_sb[:], in_=bqkv_in.rearrange("(c p) o -> p (c o)", p=128)
        )
        id64 = const.tile([128, 64], bf, tag="id64")
        nc.sync.dma_start(out=id64[:], in_=id64_in[:])
        id65 = const.tile([65, 65], f32, tag="id65")
        nc.scalar.dma_start(out=id65[:], in_=id65_in[:])

        # ---- x^T tiles (host pre-transposed as sharding prep) ----
        # two 2MB DMAs per batch on parallel queues: near line-rate head
        xTg = [
            [res.tile([128, 4, S], bf, tag=f"xTg{b}_{g}", name=f"xTg{b}_{g}") for g in range(2)]
            for b in range(B)
        ]
        x4 = xt_in.rearrange("b (g k) p s -> b g k p s", g=2)
        for b in range(B):
            for g in range(2):
                for kk in range(4):
                    eng = (nc.sync, nc.scalar)[(g + kk) % 2]
                    eng.dma_start(
                        out=xTg[b][g][:, kk, :], in_=x4[b, g, kk]
                    )

        def xs(b, k):
            return xTg[b][k // 4][:, k % 4, :]

        warm = ps_sm.tile([128, 384], f32, tag="ps1", name="warm")
        for _ in range(18):
            nc.tensor.matmul(
                warm[:], lhsT=wqkv_sb[0][:, 0:128], rhs=wqkv_sb[0][:],
                start=True, stop=True,
            )

        qT = [res.tile([128, S], bf, tag=f"qT{b}", name=f"qTt{b}") for b in range(B)]
        kT = [res.tile([128, S], bf, tag=f"kT{b}", name=f"kTt{b}") for b in range(B)]
        vT = [res.tile([128, S], bf, tag=f"vT{b}", name=f"vTt{b}") for b in range(B)]
        va = [res.tile([128, NKT, VN], bf, tag=f"va{b}", name=f"vat{b}") for b in range(B)]
        for b in range(B):
            nc.vector.memset(va[b][:], 1.0)  # ones cols survive at 64, 129

        def proj_block(b, which):
            # out[col, tok] for col-chunk `which` (0=q, 1=k, 2=v)
            dst = (qT[b], kT[b], vT[b])[which]
            for t in range(NQC):
                ps = ps_sm.tile([128, QCHUNK], f32, tag="ps1", name="projps")
                for k in range(KT):
                    nc.tensor.matmul(
                        ps[:],
                        lhsT=wqkv_sb[k][:, which * 128:(which + 1) * 128],
                        rhs=xs(b, k)[:, t * QCHUNK:(t + 1) * QCHUNK],
                        start=(k == 0),
                        stop=(k == KT - 1),
                    )
                nc.vector.tensor_scalar_add(
                    out=dst[:, t * QCHUNK:(t + 1) * QCHUNK],
                    in0=ps[:],
                    scalar1=bqkv_sb[:, which:which + 1],
                )

        def v_transposes(b, t):
            # vT chunk t covers kpos tiles 4t..4t+3; transpose to va natural
            for kt in range(4 * t, 4 * t + 4):
                for h in range(HPC):
                    pst = ps_sm.tile([128, 64], bf, tag="ps1", name="vtps")
                    nc.tensor.transpose(
                        pst[:],
                        in_=vT[b][h * 64:(h + 1) * 64, kt * 128:(kt + 1) * 128],
                        identity=id64[h * 64:(h + 1) * 64, :],
                    )
                    nc.vector.tensor_copy(
                        out=va[b][:, kt, h * (D + 1):h * (D + 1) + D], in_=pst[:]
                    )

        def proj_pump(b, whiches=(2, 1, 0)):
            # yields after each emitted op so attend() can interleave
            # projection work into PE idle slots of the ACT-paced attention
            for which in whiches:
                dst = (qT[b], kT[b], vT[b])[which]
                for t in range(NQC):
                    ps = ps_sm.tile([128, QCHUNK], f32, tag="ps1", name="projps")
                    for k in range(KT):
                        nc.tensor.matmul(
                            ps[:],
                            lhsT=wqkv_sb[k][:, which * 128:(which + 1) * 128],
                            rhs=xs(b, k)[:, t * QCHUNK:(t + 1) * QCHUNK],
                            start=(k == 0),
                            stop=(k == KT - 1),
                        )
                        yield
                    nc.vector.tensor_scalar_add(
                        out=dst[:, t * QCHUNK:(t + 1) * QCHUNK],
                        in0=ps[:],
                        scalar1=bqkv_sb[:, which:which + 1],
                    )
                    yield
                    if which == 2:
                        for kt in range(4 * t, 4 * t + 4):
                            for h in range(HPC):
                                pst = ps_sm.tile([128, 64], bf, tag="ps1", name="vtps")
                                nc.tensor.transpose(
                                    pst[:],
                                    in_=vT[b][h * 64:(h + 1) * 64, kt * 128:(kt + 1) * 128],
                                    identity=id64[h * 64:(h + 1) * 64, :],
                                )
                                nc.vector.tensor_copy(
                                    out=va[b][:, kt, h * (D + 1):h * (D + 1) + D],
                                    in_=pst[:],
                                )
                            yield

        def _norm_unit(att_sb, og, h, qs):
            pst = ps_sm.tile([128, D + 1], f32, tag="ps1", name="attt")
            nc.tensor.transpose(
                pst[:], in_=att_sb[:, qs * 128:(qs + 1) * 128], identity=id65[:]
            )
            rec = rp.tile([128, 1], f32, tag="rec", name="rec")
            nc.vector.reciprocal(out=rec[:], in_=pst[:, D:D + 1])
            nc.vector.tensor_scalar_mul(
                out=og[:, qs, h * D:(h + 1) * D], in0=pst[:, 0:D], scalar1=rec[:]
            )

        def attend(b, qc, pump=None, rate=3, pending=None, last=False):
            og = ogp.tile([128, 4, 2 * D], f32, tag="og", name="og")
            # attn^T accumulators per head: rows 0-63 = dims, row 64 = sum(exp)
            att = [
                ps_sm.tile([D + 1, QCHUNK], f32, tag="ps1", name="attps")
                for _ in range(HPC)
            ]
            qsl = qT[b][:, qc * QCHUNK:(qc + 1) * QCHUNK]
            for kt in range(NKT):
                # both heads' scores for this kpos tile in one 2-bank tile;
                # alternating PE row groups (0 / 64) hide each LDWEIGHTS
                # under the other head's matmul
                sc = ps_sc.tile([128, 1024], f32, tag="sc", name="scps")
                for h in range(HPC):
                    nc.tensor.matmul(
                        sc[:, h * 512:(h + 1) * 512],
                        lhsT=kT[b][h * 64:(h + 1) * 64, kt * 128:(kt + 1) * 128],
                        rhs=qsl[h * 64:(h + 1) * 64, :],
                        start=True,
                        stop=True,
                    )
                ex = expp.tile([128, 1024], bf, tag="ex", name="ex")
                nc.scalar.activation(
                    out=ex[:], in_=sc[:], func=mybir.ActivationFunctionType.Exp
                )
                for h in range(HPC):
                    nc.tensor.matmul(
                        att[h][:],
                        lhsT=va[b][:, kt, h * (D + 1):(h + 1) * (D + 1)],
                        rhs=ex[:, h * 512:(h + 1) * 512],
                        start=(kt == 0),
                        stop=(kt == NKT - 1),
                    )
                if pending:
                    for _ in range(2):
                        if pending:
                            pending.pop(0)()
                if pump is not None:
                    for _ in range(rate):
                        next(pump, None)
            while pending:
                pending.pop(0)()
            # release the PSUM accumulators now; defer the PE/DVE normalize
            # tail into the next attend's kt loop so it doesn't stall ScalarE
            new_pending = []
            for h in range(HPC):
                att_sb = atn.tile([D + 1, QCHUNK], f32, tag="atn", name="attsb")
                nc.vector.tensor_copy(out=att_sb[:], in_=att[h][:])
                for qs in range(4):
                    new_pending.append(
                        lambda a=att_sb, hh=h, q=qs: _norm_unit(a, og, hh, q)
                    )
            new_pending.append(
                lambda: nc.gpsimd.dma_start(
                    out=out_d.rearrange("(qs p) b e -> p qs b e", p=128)[
                        :, qc * 4:(qc + 1) * 4, b, :
                    ],
                    in_=og[:],
                )
            )
            if last:
                for th in new_pending:
                    th()
                return []
            return new_pending

        # b0: only k and q projections up front (all the exp chain needs);
        # v projection + transposes dribble into attend(0,0)'s scores/exp
        # phase so ScalarE starts ~25us earlier. Tile deps are emission-
        # ordered, so attend(0,0)'s attT matmuls (which consume va) are
        # emitted only AFTER the v pump is fully drained.
        proj_block(0, 1)
        proj_block(0, 0)
        pump0 = proj_pump(0, whiches=(2,))
        og0 = ogp.tile([128, 4, 2 * D], f32, tag="og", name="og")
        att0 = [
            ps_sm.tile([D + 1, QCHUNK], f32, tag="ps1", name="attps")
            for _ in range(HPC)
        ]
        exs0 = []
        for kt in range(NKT):
            sc = ps_sc.tile([128, 1024], f32, tag="sc", name="scps")
            for h in range(HPC):
                nc.tensor.matmul(
                    sc[:, h * 512:(h + 1) * 512],
                    lhsT=kT[0][h * 64:(h + 1) * 64, kt * 128:(kt + 1) * 128],
                    rhs=qT[0][h * 64:(h + 1) * 64, 0:QCHUNK],
                    start=True,
                    stop=True,
                )
            ex = expp.tile([128, 1024], bf, tag="ex", name="ex")
            nc.scalar.activation(
                out=ex[:], in_=sc[:], func=mybir.ActivationFunctionType.Exp
            )
            exs0.append(ex)
            for _ in range(4):
                next(pump0, None)
        for _ in pump0:
            pass
        for kt in range(NKT):
            for h in range(HPC):
                nc.tensor.matmul(
                    att0[h][:],
                    lhsT=va[0][:, kt, h * (D + 1):(h + 1) * (D + 1)],
                    rhs=exs0[kt][:, h * 512:(h + 1) * 512],
                    start=(kt == 0),
                    stop=(kt == NKT - 1),
                )
        pend = []
        for h in range(HPC):
            att_sb = atn.tile([D + 1, QCHUNK], f32, tag="atn", name="attsb")
            nc.vector.tensor_copy(out=att_sb[:], in_=att0[h][:])
            for qs in range(4):
                pend.append(lambda a=att_sb, hh=h, q=qs: _norm_unit(a, og0, hh, q))
        pend.append(
            lambda: nc.gpsimd.dma_start(
                out=out_d.rearrange("(qs p) b e -> p qs b e", p=128)[:, 0:4, 0, :],
                in_=og0[:],
            )
        )
        # b1 projection dribbled into the remaining attend(b0) chunks;
        # each attend's normalize tail is deferred into the next attend
        pump = proj_pump(1)
        for qc in range(1, NQC):
            pend = attend(0, qc, pump, pending=pend)
        for _ in pump:
            pass
        for qc in range(NQC):
            pend = attend(1, qc, pending=pend, last=(qc == NQC - 1))

    nc.compile()
    return nc


def _get_nc():
    if "nc" not in _BUILT:
        _BUILT["nc"] = _build_bass()
    return _BUILT["nc"]


def _prep_core_inputs(x_bf, W, b):
    """Per-core input dicts. W/b slicing+scaling+casting is host-side weight prep."""
    _id64 = np.concatenate([np.eye(64), np.eye(64)], axis=0).astype(_BF16)
    _id65 = np.eye(65, dtype=np.float32)
    in_maps = []
    for c in range(NCORES):
        q0 = 2 * c * D          # first col of this core's head pair
        wq = W[:, q0:q0 + 128] * SCALING
        wk = W[:, E + q0:E + q0 + 128]
        wv = W[:, 2 * E + q0:2 * E + q0 + 128]
        wqkv = np.concatenate([wq, wk, wv], axis=1).astype(_BF16)
        bqkv = np.concatenate(
            [b[q0:q0 + 128] * SCALING, b[E + q0:E + q0 + 128],
             b[2 * E + q0:2 * E + q0 + 128]]
        ).astype(np.float32)[:, None]
        in_maps.append(
            {
                "xt": x_bf,
                "wqkv": np.ascontiguousarray(wqkv),
                "bqkv": np.ascontiguousarray(bqkv),
                "id64": _id64,
                "id65": _id65,
            }
        )
    return in_maps


def run(inputs, trace=False):
    """Returns (output [S,B,E] fp32, BassKernelResults)."""
    from concourse.bass_utils import run_bass_kernel_spmd

    x = np.asarray(inputs["x"], np.float32)
    W = np.asarray(inputs["W_in"], np.float32)
    b = np.asarray(inputs["b_in"], np.float32)
    # sharding prep: cast + de-interleave batches + transpose to [B, KT, 128, S]
    x_bf = np.ascontiguousarray(
        x.reshape(S, B, KT, 128).transpose(1, 2, 3, 0)
    ).astype(_BF16)

    nc = _get_nc()
    in_maps = _prep_core_inputs(x_bf, W, b)
    res = run_bass_kernel_spmd(
        nc, in_maps, core_ids=list(range(NCORES)), trace=trace
    )
    out = np.concatenate([r["out"] for r in res.results], axis=2)
    return out, res


def kernel(**inputs):
    out, _ = run(inputs, trace=False)
    return out
